# revision 10
# baseline (speedup 1.0000x reference)
"""Multi-head causal attention (B=4, L=2048, D=1024, H=16) on 8 TRN2 NeuronCores.

Sharding: core c handles batch b = c//2 and head-group hg = c%2 (8 heads, 512 dims).
Each core computes Q/K/V projections for its heads, causal attention, and a
partial output projection (its 512 input dims of Wo). Host sums the two
partials per batch.

Matmuls run in fp16 (same PE rate as bf16, better mantissa; ~1e-3 rel err).
Structure keeps the PE busy through the attention phase (Q-projection of the
next q-chunk and output-projection of the previous q-chunk are interleaved
with S/AV matmuls) so the HAM activity monitor holds the 2.4 GHz clock.
"""
import sys

sys.path.insert(0, "/opt/trn_rl_repo")

import numpy as np

import concourse.bass as bass
import concourse.mybir as mybir
import concourse.tile as tile
from concourse import bacc
from concourse.masks import make_identity

F32 = mybir.dt.float32
F16 = mybir.dt.float16
MM = F16
AF = mybir.ActivationFunctionType

B, L, D, H = 4, 2048, 1024, 16
DK = 64          # head dim
E = 512          # per-core head dims (8 heads)
NL = L // 128    # 16 l-tiles
ND = D // 128    # 8 d-tiles (contraction for projections)
NE = E // 128    # 4 e-tiles
NJ = L // 512    # 4 q-chunks
NK = L // 128    # 16 k-tiles
NDO = E // 128   # 4 d-tiles for out-proj contraction

PT_BUFS = 2

_CACHE = {}


def build_program():
    nc = bacc.Bacc("TRN2", target_bir_lowering=False, debug=False, num_devices=8)

    xb = nc.dram_tensor("xb", [L, D], F32, kind="ExternalInput")
    wq = nc.dram_tensor("wq", [E, D], F32, kind="ExternalInput")
    wk = nc.dram_tensor("wk", [E, D], F32, kind="ExternalInput")
    wv = nc.dram_tensor("wv", [E, D], F32, kind="ExternalInput")
    wo = nc.dram_tensor("wo", [D, E], F32, kind="ExternalInput")
    masks = nc.dram_tensor("masks", [4, 128, 512], F16, kind="ExternalInput")
    out = nc.dram_tensor("out", [L, D], F32, kind="ExternalOutput")

    with tile.TileContext(nc) as tc:
        with (
            tc.tile_pool(name="const", bufs=1) as constp,
            tc.tile_pool(name="big", bufs=1) as bigp,
            tc.tile_pool(name="stripp", bufs=2) as stripp,
            tc.tile_pool(name="qtc", bufs=2) as qtcp,
            tc.tile_pool(name="ptp", bufs=PT_BUFS) as ptp,
            tc.tile_pool(name="smallp", bufs=2) as smallp,
            tc.tile_pool(name="psX", bufs=2, space="PSUM") as psX,
            tc.tile_pool(name="psP", bufs=2, space="PSUM") as psP,
            tc.tile_pool(name="psS", bufs=2, space="PSUM") as psS,
            tc.tile_pool(name="psAV", bufs=2, space="PSUM") as psAV,
        ):
            ident = constp.tile([128, 128], F32)
            make_identity(nc, ident[:])
            ident_h = constp.tile([128, 128], F16)
            make_identity(nc, ident_h[:])
            ones_f = constp.tile([128, 16], F32)
            nc.vector.memset(ones_f[:], 1.0)
            ones_r = constp.tile([1, 64], MM)
            nc.vector.tensor_copy(ones_r[:], ones_f[0:1, 0:1].to_broadcast((1, 64)))
            masks_sb = constp.tile([128, 4, 512], F16)
            for m in range(4):
                nc.sync.dma_start(masks_sb[:, m, :], masks[m])

            xT = bigp.tile([128, ND, L], MM)          # x^T, [d-in-tile, d-tile, l]
            WTq = bigp.tile([128, ND, E], MM)
            WTk = bigp.tile([128, ND, E], MM)
            WTv = bigp.tile([128, ND, E], MM)
            KT = bigp.tile([128, NE, L], MM)
            VT = bigp.tile([128, NE, L], MM)
            attT = bigp.tile([128, NDO, L], MM)
            WoT = bigp.tile([128, NDO, D], MM)
            Vaug = bigp.tile([128, 8, NK, 65], MM)    # per-head V [k, dk] + ones col

            # ---- load + transpose weights ----
            for wdram, WT in ((wq, WTq), (wk, WTk), (wv, WTv)):
                for et in range(NE):
                    strip = stripp.tile([128, D], F32, tag="strip")
                    nc.sync.dma_start(strip[:], wdram[et * 128:(et + 1) * 128, :])
                    for dt in range(ND):
                        tp = psX.tile([128, 128], F32, tag="x", name="tp")
                        nc.tensor.transpose(
                            tp[:], strip[:, dt * 128:(dt + 1) * 128], ident[:]
                        )
                        nc.vector.tensor_copy(WT[:, dt, et * 128:(et + 1) * 128], tp[:])
            for et8 in range(ND):
                strip = stripp.tile([128, D], F32, tag="strip")
                nc.sync.dma_start(strip[:, 0:E], wo[et8 * 128:(et8 + 1) * 128, :])
                for dt in range(NDO):
                    tp = psX.tile([128, 128], F32, tag="x", name="tp")
                    nc.tensor.transpose(
                        tp[:], strip[:, dt * 128:(dt + 1) * 128], ident[:]
                    )
                    nc.vector.tensor_copy(WoT[:, dt, et8 * 128:(et8 + 1) * 128], tp[:])

            # ---- load + transpose x ----
            for lt in range(NL):
                strip = stripp.tile([128, D], F32, tag="strip")
                nc.sync.dma_start(strip[:], xb[lt * 128:(lt + 1) * 128, :])
                for dt in range(ND):
                    tp = psX.tile([128, 128], F32, tag="x", name="tp")
                    nc.tensor.transpose(
                        tp[:], strip[:, dt * 128:(dt + 1) * 128], ident[:]
                    )
                    nc.vector.tensor_copy(xT[:, dt, lt * 128:(lt + 1) * 128], tp[:])

            # ---- K/V projections (full L) ----
            for WT, OUT in ((WTk, KT), (WTv, VT)):
                for et in range(NE):
                    for jc in range(NJ):
                        pp = psP.tile([128, 512], F32, tag="pp")
                        for dt in range(ND):
                            nc.tensor.matmul(
                                pp[:],
                                WT[:, dt, et * 128:(et + 1) * 128],
                                xT[:, dt, jc * 512:(jc + 1) * 512],
                                start=(dt == 0),
                                stop=(dt == ND - 1),
                            )
                        nc.vector.tensor_copy(
                            OUT[:, et, jc * 512:(jc + 1) * 512], pp[:]
                        )

            # ---- V^T -> V natural (all heads) + ones column ----
            nc.vector.tensor_copy(
                Vaug[:, :, :, 64:65],
                ones_f[:, 0:1].to_broadcast((128, 8, NK, 1)),
            )
            for h in range(8):
                hp = (h % 2) * 64
                hb = h // 2
                for kt in range(NK):
                    tpv = psX.tile([128, 64], F16, tag="x", name="tpv")
                    nc.tensor.transpose(
                        tpv[:],
                        VT[hp:hp + 64, hb, kt * 128:(kt + 1) * 128],
                        ident_h[hp:hp + 64, hp:hp + 64],
                    )
                    nc.vector.tensor_copy(Vaug[:, h, kt, 0:64], tpv[:])

            def q_proj(j, qtile):
                for et in range(NE):
                    pp = psP.tile([128, 512], F32, tag="pp")
                    for dt in range(ND):
                        nc.tensor.matmul(
                            pp[:],
                            WTq[:, dt, et * 128:(et + 1) * 128],
                            xT[:, dt, j * 512:(j + 1) * 512],
                            start=(dt == 0),
                            stop=(dt == ND - 1),
                        )
                    nc.vector.tensor_copy(qtile[:, et, :], pp[:])

            def out_proj(j):
                for lt in range(4 * j, 4 * j + 4):
                    for ec in range(2):
                        op = psP.tile([128, 512], F32, tag="pp", name="op")
                        for dt in range(NDO):
                            nc.tensor.matmul(
                                op[:],
                                attT[:, dt, lt * 128:(lt + 1) * 128],
                                WoT[:, dt, ec * 512:(ec + 1) * 512],
                                start=(dt == 0),
                                stop=(dt == NDO - 1),
                            )
                        ot = smallp.tile([128, 512], F32, tag="ot")
                        nc.vector.tensor_copy(ot[:], op[:])
                        nc.sync.dma_start(
                            out[lt * 128:(lt + 1) * 128, ec * 512:(ec + 1) * 512],
                            ot[:],
                        )

            # ---- attention, q-chunk outer / head inner ----
            qtiles = {}
            qtiles[0] = qtcp.tile([128, NE, 512], MM, tag="qt", name="qt0")
            q_proj(0, qtiles[0])
            for j in range(NJ):
                if j + 1 < NJ:
                    qtiles[j + 1] = qtcp.tile([128, NE, 512], MM, tag="qt", name=f"qt{j+1}")
                    q_proj(j + 1, qtiles[j + 1])
                nkt = 4 * (j + 1)
                QTc = qtiles[j]
                for h in range(8):
                    hp = (h % 2) * 64
                    hb = h // 2
                    PT = ptp.tile([128, NK, 512], MM, tag="pt")
                    for kt in range(nkt):
                        s_ps = psS.tile([128, 512], F32, tag="s")
                        nc.tensor.matmul(
                            s_ps[:],
                            KT[hp:hp + 64, hb, kt * 128:(kt + 1) * 128],
                            QTc[hp:hp + 64, hb, :],
                        )
                        nc.scalar.activation(
                            PT[:, kt, :], s_ps[:], AF.Exp, scale=0.125
                        )
                        if kt >= nkt - 4:
                            nc.vector.tensor_mul(
                                PT[:, kt, :],
                                PT[:, kt, :],
                                masks_sb[:, kt - (nkt - 4), :],
                            )
                    att_ps = psAV.tile([65, 512], F32, tag="av")
                    for kt in range(nkt):
                        nc.tensor.matmul(
                            att_ps[:],
                            Vaug[:, h, kt, 0:65],
                            PT[:, kt, :],
                            start=(kt == 0),
                            stop=(kt == nkt - 1),
                        )
                    recip = smallp.tile([1, 512], MM, tag="recip")
                    with nc.allow_low_precision(reason="fp16 rounding for PE broadcast"):
                        nc.vector.reciprocal(recip[:], att_ps[64:65, :])
                    bc_ps = psX.tile([64, 512], F32, tag="x", name="bc_ps")
                    nc.tensor.matmul(bc_ps[:], ones_r[:], recip[:])
                    bc_sb = smallp.tile([64, 512], F32, tag="bcsb")
                    nc.vector.tensor_copy(bc_sb[:], bc_ps[:])
                    nc.vector.tensor_mul(
                        attT[hp:hp + 64, hb, j * 512:(j + 1) * 512],
                        att_ps[0:64, :],
                        bc_sb[:],
                    )
                out_proj(j)

    nc.compile()
    return nc


def build_masks():
    kp = np.arange(128)[:, None]
    qf = np.arange(512)[None, :]
    return np.stack(
        [(qf >= kp + 128 * m).astype(np.float16) for m in range(4)]
    )


def _get_program():
    if "nc" not in _CACHE:
        _CACHE["nc"] = build_program()
    return _CACHE["nc"]


def make_in_maps(x, Wq, Wk, Wv, Wo):
    x = np.asarray(x, dtype=np.float32)
    Wq = np.asarray(Wq, dtype=np.float32)
    Wk = np.asarray(Wk, dtype=np.float32)
    Wv = np.asarray(Wv, dtype=np.float32)
    Wo = np.asarray(Wo, dtype=np.float32)
    masks = build_masks()
    in_maps = []
    for c in range(8):
        b, hg = c // 2, c % 2
        sl = slice(hg * E, (hg + 1) * E)
        in_maps.append(
            {
                "xb": np.ascontiguousarray(x[b]),
                "wq": np.ascontiguousarray(Wq[sl]),
                "wk": np.ascontiguousarray(Wk[sl]),
                "wv": np.ascontiguousarray(Wv[sl]),
                "wo": np.ascontiguousarray(Wo[:, sl]),
                "masks": masks,
            }
        )
    return in_maps


def kernel(x, Wq, Wk, Wv, Wo, **run_kwargs):
    from concourse import bass_utils

    nc = _get_program()
    in_maps = make_in_maps(x, Wq, Wk, Wv, Wo)
    res = bass_utils.run_bass_kernel_spmd(
        nc, in_maps, core_ids=list(range(8)), **run_kwargs
    )
    out = np.empty((B, L, D), np.float32)
    for b in range(B):
        out[b] = res.results[2 * b]["out"] + res.results[2 * b + 1]["out"]
    _CACHE["last_result"] = res
    return out
